# revision 13
# baseline (speedup 1.0000x reference)
"""BiAttention (BiDAF-style) layer for Trainium2, data-parallel over batch.

Shapes (hardcoded, from the problem spec):
  encoded_passage  [B=8, P=2048, D=768] f32
  encoded_question [B=8, Q=256,  D=768] f32
  passage_mask     [B=8, P=2048] f32 (binary)
  question_mask    [B=8, Q=256]  f32 (binary)
  output           [B=8, P=2048, 4*D=3072] f32

Each of the 8 NeuronCores processes one batch element; no communication.
"""

import numpy as np

B, P, Q, D = 8, 2048, 256, 768
N_CORES = 8
EPS = 1e-07
NEG_VAL = -10000000.0

NT = P // 128  # 16 passage tiles
DC = D // 128  # 6 contraction chunks
QC = Q // 128  # 2 question chunks

# float32r matmuls run at full PE rate for N>=256 (fp32 is 1/4 rate) at
# reduced multiply precision (tf32-like). Walrus requires every producer of an
# fp32r-matmul operand to emit fp32r, so the operand tiles are allocated as
# fp32r and exact-fp32 consumers read them through a bitcast view.
USE_F32R = True


def build_nc():
    """Build (trace + schedule + bacc-compile) the single-core Bass program."""
    import concourse.bass as bass
    import concourse.mybir as mybir
    import concourse.tile as tile
    import concourse.bass_isa as bass_isa
    from concourse import bacc
    from concourse.bass import ts
    from concourse.masks import make_identity

    f32 = mybir.dt.float32
    f32r = mybir.dt.float32r
    Alu = mybir.AluOpType
    Act = mybir.ActivationFunctionType
    Axis = mybir.AxisListType

    nc = bacc.Bacc(
        "TRN2",
        target_bir_lowering=False,
        debug=False,
        enable_asserts=False,
        num_devices=N_CORES,
    )

    ep = nc.dram_tensor("encoded_passage", [P, D], f32, kind="ExternalInput").ap()
    eq = nc.dram_tensor("encoded_question", [Q, D], f32, kind="ExternalInput").ap()
    pmsk = nc.dram_tensor("passage_mask", [P], f32, kind="ExternalInput").ap()
    qmsk = nc.dram_tensor("question_mask", [Q], f32, kind="ExternalInput").ap()
    out = nc.dram_tensor("out", [P, 4 * D], f32, kind="ExternalOutput").ap()

    dt_mm = f32r if USE_F32R else f32

    with tile.TileContext(nc) as tc:
        with (
            tc.tile_pool(name="const", bufs=1) as const,
            tc.tile_pool(name="work", bufs=2) as work,
            tc.tile_pool(name="sm", bufs=3) as sm,
            tc.tile_pool(name="small", bufs=3) as small,
            tc.tile_pool(name="store", bufs=3) as store,
            tc.tile_pool(name="psA", bufs=2, space="PSUM") as psA,
            tc.tile_pool(name="psB", bufs=2, space="PSUM") as psB,
        ):
            # ---- constants / persistent tiles ----
            ident = const.tile([128, 128], f32)
            make_identity(nc, ident)

            pas_all = const.tile([128, NT, D], dt_mm)  # passage, natural layout
            qnat = const.tile([128, QC, D], dt_mm)  # question, natural layout
            qT = const.tile([128, DC, Q], dt_mm)  # question transposed [d, q]
            qmask_b = const.tile([128, Q], f32)  # question mask bcast over rows
            negq_b = const.tile([128, Q], f32)  # NEG_VAL * (1 - qmask)
            qp_all = const.tile([128, NT], f32)  # qp_similarity columns
            r_all = const.tile([128, NT], f32)  # 1/(softmax sum + eps) per tile
            qp_bc = const.tile([128, D], f32)  # qp_vector broadcast

            # question: load natural, broadcast masks
            for qc in range(QC):
                nc.sync.dma_start(
                    out=qnat[:, qc, :], in_=eq[ts(qc, 128), :].bitcast(dt_mm)
                )
            nc.sync.dma_start(out=qmask_b[:, :], in_=qmsk.partition_broadcast(128))
            nc.vector.tensor_scalar(
                out=negq_b[:, :],
                in0=qmask_b[:, :],
                scalar1=-NEG_VAL,
                scalar2=NEG_VAL,
                op0=Alu.mult,
                op1=Alu.add,
            )

            # question transpose: qT[:, dc, qc*128:(qc+1)*128] = qnat[:, qc, dc].T
            qnat_f32 = qnat.bitcast(f32)
            for dc in range(DC):
                ps_q = psA.tile([128, 256], f32, tag="tr")
                for qc in range(QC):
                    nc.tensor.transpose(
                        ps_q[:, ts(qc, 128)],
                        qnat_f32[:, qc, ts(dc, 128)],
                        ident[:, :],
                    )
                nc.vector.tensor_copy(qT[:, dc, :], ps_q[:, :])

            # passage mask, transposed to [p_in_tile, tile] layout
            pm_nat = const.tile([16, 128], f32)
            nc.sync.dma_start(
                out=pm_nat[:, :], in_=pmsk.rearrange("(t p) -> t p", p=128)
            )
            ps_pm = psA.tile([128, 256], f32, tag="tr")
            nc.tensor.transpose(ps_pm[:, 0:16], pm_nat[:, :], ident[0:16, 0:16])
            pm_t = const.tile([128, NT], f32)
            nc.vector.tensor_copy(pm_t[:, :], ps_pm[:, 0:16])

            # ---- phase 1: per passage-tile attention ----
            for t in range(NT):
                # load passage tile (kept resident for phases 2/3)
                nc.sync.dma_start(
                    out=pas_all[:, t, :], in_=ep[ts(t, 128), :].bitcast(dt_mm)
                )

                # transpose passage tile: pT[:, dc, :] = pas[:, dc-chunk].T
                pas_f32 = pas_all.bitcast(f32)
                pT = work.tile([128, DC, 128], dt_mm, tag="pT")
                for h in range(DC // 2):
                    ps_tr = psA.tile([128, 256], f32, tag="tr")
                    for j in range(2):
                        dc = 2 * h + j
                        nc.tensor.transpose(
                            ps_tr[:, ts(j, 128)],
                            pas_f32[:, t, ts(dc, 128)],
                            ident[:, :],
                        )
                    nc.vector.tensor_copy(pT[:, 2 * h : 2 * h + 2, :], ps_tr[:, :])

                # sim tile [128, Q] = passage_tile @ question^T
                ps_sim = psA.tile([128, Q], f32, tag="sim")
                for dc in range(DC):
                    nc.tensor.matmul(
                        ps_sim[:, :],
                        lhsT=pT[:, dc, :],
                        rhs=qT[:, dc, :],
                        start=(dc == 0),
                        stop=(dc == DC - 1),
                    )

                # im = sim * qmask; negm1 = -max(im)
                im = sm.tile([128, Q], f32, tag="im")
                negm1 = small.tile([128, 1], f32, tag="negm1")
                nc.vector.tensor_mul(im[:, :], ps_sim[:, :], qmask_b[:, :])
                nc.vector.tensor_reduce(
                    out=negm1[:, :], in_=im[:, :], axis=Axis.X, op=Alu.max, negate=True
                )
                # masked_sim = sim*qmask + NEG_VAL*(1-qmask)
                msim = sm.tile([128, Q], f32, tag="msim")
                nc.vector.tensor_add(msim[:, :], im[:, :], negq_b[:, :])
                # qp_similarity column for this tile
                nc.vector.tensor_reduce(
                    out=qp_all[:, t : t + 1],
                    in_=msim[:, :],
                    axis=Axis.X,
                    op=Alu.max,
                )
                # t' = exp(masked_sim - m1)  (masked entries underflow to 0)
                tprime = sm.tile([128, Q], f32, tag="tp")
                ssum = small.tile([128, 1], f32, tag="ssum")
                nc.scalar.activation(
                    out=tprime[:, :],
                    in_=msim[:, :],
                    func=Act.Exp,
                    bias=negm1[:, :],
                    scale=1.0,
                    accum_out=ssum[:, :],
                )
                se = small.tile([128, 1], f32, tag="se")
                nc.vector.tensor_scalar_add(se[:, :], ssum[:, :], EPS)
                nc.vector.reciprocal(r_all[:, t : t + 1], se[:, :])

                # transpose t' -> [q, p] for the pq matmul
                ps_tr2 = psA.tile([128, 256], f32, tag="tr")
                for qc in range(QC):
                    nc.tensor.transpose(
                        ps_tr2[:, ts(qc, 128)],
                        tprime[:, ts(qc, 128)],
                        ident[:, :],
                    )
                aT = work.tile([128, QC, 128], dt_mm, tag="aT")
                nc.scalar.copy(aT[:, :, :], ps_tr2[:, :])

                # pq_vectors tile [128, D] = t'^T.T @ question  (unnormalized)
                ps_pq = psB.tile([128, D], f32, tag="pq")
                for qc in range(QC):
                    st = qc == 0
                    sp = qc == QC - 1
                    nc.tensor.matmul(
                        ps_pq[:, 0:512],
                        lhsT=aT[:, qc, :],
                        rhs=qnat[:, qc, 0:512],
                        start=st,
                        stop=sp,
                    )
                    nc.tensor.matmul(
                        ps_pq[:, 512:D],
                        lhsT=aT[:, qc, :],
                        rhs=qnat[:, qc, 512:D],
                        start=st,
                        stop=sp,
                    )

                # evict + normalize pq, form passage*pq, store out cols 768:2304
                pas_f32 = pas_all.bitcast(f32)
                o23 = store.tile([128, 2 * D], f32, tag="o23")
                nc.scalar.mul(o23[:, 0:D], ps_pq[:, :], r_all[:, t : t + 1])
                nc.gpsimd.tensor_mul(o23[:, D : 2 * D], pas_f32[:, t, :], o23[:, 0:D])
                nc.sync.dma_start(out=out[ts(t, 128), 0:D], in_=pas_f32[:, t, :])
                nc.sync.dma_start(out=out[ts(t, 128), D : 3 * D], in_=o23[:, :])

            # ---- phase 2: masked softmax over all P, then qp_vector ----
            neg2 = const.tile([128, NT], f32)
            nc.vector.tensor_scalar(
                out=neg2[:, :],
                in0=pm_t[:, :],
                scalar1=-NEG_VAL,
                scalar2=NEG_VAL,
                op0=Alu.mult,
                op1=Alu.add,
            )
            im2 = const.tile([128, NT], f32)
            nc.vector.tensor_mul(im2[:, :], qp_all[:, :], pm_t[:, :])
            rowmax = small.tile([128, 1], f32, tag="p2")
            nc.vector.tensor_reduce(
                out=rowmax[:, :], in_=im2[:, :], axis=Axis.X, op=Alu.max
            )
            gmax = small.tile([128, 1], f32, tag="p2")
            nc.gpsimd.partition_all_reduce(
                gmax[:, :], rowmax[:, :], channels=128, reduce_op=bass_isa.ReduceOp.max
            )
            neggmax = small.tile([128, 1], f32, tag="p2")
            nc.vector.tensor_scalar_mul(neggmax[:, :], gmax[:, :], -1.0)

            ms2 = const.tile([128, NT], f32)
            nc.vector.tensor_add(ms2[:, :], im2[:, :], neg2[:, :])
            t2 = const.tile([128, NT], f32)
            s2row = small.tile([128, 1], f32, tag="p2")
            nc.scalar.activation(
                out=t2[:, :],
                in_=ms2[:, :],
                func=Act.Exp,
                bias=neggmax[:, :],
                scale=1.0,
                accum_out=s2row[:, :],
            )
            gsum = small.tile([128, 1], f32, tag="p2")
            nc.gpsimd.partition_all_reduce(
                gsum[:, :], s2row[:, :], channels=128, reduce_op=bass_isa.ReduceOp.add
            )
            ser2 = small.tile([128, 1], f32, tag="p2")
            nc.vector.tensor_scalar_add(ser2[:, :], gsum[:, :], EPS)
            r2 = small.tile([128, 1], f32, tag="p2")
            nc.vector.reciprocal(r2[:, :], ser2[:, :])
            attn2 = const.tile([128, NT], dt_mm)
            nc.vector.tensor_scalar_mul(attn2[:, :], t2[:, :], r2[:, :])

            # qp_vector[d] = sum_p attn2[p] * passage[p, d]
            ps_qp = psB.tile([1, D], f32, tag="pq")
            for t in range(NT):
                st = t == 0
                sp = t == NT - 1
                nc.tensor.matmul(
                    ps_qp[:, 0:512],
                    lhsT=attn2[:, t : t + 1],
                    rhs=pas_all[:, t, 0:512],
                    start=st,
                    stop=sp,
                )
                nc.tensor.matmul(
                    ps_qp[:, 512:D],
                    lhsT=attn2[:, t : t + 1],
                    rhs=pas_all[:, t, 512:D],
                    start=st,
                    stop=sp,
                )
            qp_sb = const.tile([1, D], f32)
            nc.scalar.copy(qp_sb[:, :], ps_qp[:, :])
            nc.gpsimd.partition_broadcast(qp_bc[:, :], qp_sb[:, :], channels=128)

            # ---- phase 3: passage * qp_vector, store out cols 2304:3072 ----
            pas_f32 = pas_all.bitcast(f32)
            for g in range(NT // 4):
                big = store.tile([128, 4, D], f32, tag="o4")
                for j in range(4):
                    t = 4 * g + j
                    eng = nc.vector if j % 2 == 0 else nc.gpsimd
                    eng.tensor_mul(big[:, j, :], pas_f32[:, t, :], qp_bc[:, :])
                dst = out[g * 512 : (g + 1) * 512, 3 * D : 4 * D].rearrange(
                    "(tt p) c -> p tt c", p=128
                )
                nc.sync.dma_start(out=dst, in_=big[:, :, :])

    nc.compile()
    return nc


_NC_CACHE = {}


def _get_nc():
    if "nc" not in _NC_CACHE:
        _NC_CACHE["nc"] = build_nc()
    return _NC_CACHE["nc"]


def kernel(
    encoded_passage: np.ndarray,
    encoded_question: np.ndarray,
    passage_mask: np.ndarray,
    question_mask: np.ndarray,
) -> np.ndarray:
    from concourse.bass_utils import run_bass_kernel_spmd

    nc = _get_nc()
    in_maps = [
        {
            "encoded_passage": np.ascontiguousarray(
                encoded_passage[b], dtype=np.float32
            ),
            "encoded_question": np.ascontiguousarray(
                encoded_question[b], dtype=np.float32
            ),
            "passage_mask": np.ascontiguousarray(passage_mask[b], dtype=np.float32),
            "question_mask": np.ascontiguousarray(question_mask[b], dtype=np.float32),
        }
        for b in range(B)
    ]
    res = run_bass_kernel_spmd(nc, in_maps, core_ids=list(range(N_CORES)))
    return np.stack([res.results[b]["out"] for b in range(B)], axis=0)


# revision 34
# speedup vs baseline: 1.2212x; 1.2212x over previous
"""BiAttention (BiDAF-style) layer for Trainium2, data-parallel over batch.

Shapes (hardcoded, from the problem spec):
  encoded_passage  [B=8, P=2048, D=768] f32
  encoded_question [B=8, Q=256,  D=768] f32
  passage_mask     [B=8, P=2048] f32 (binary)
  question_mask    [B=8, Q=256]  f32 (binary)
  output           [B=8, P=2048, 4*D=3072] f32

Each of the 8 NeuronCores processes one batch element; no communication.
"""

import numpy as np

B, P, Q, D = 8, 2048, 256, 768
N_CORES = 8
EPS = 1e-07
NEG_VAL = -10000000.0

NT = P // 128  # 16 passage tiles
DC = D // 128  # 6 contraction chunks
QC = Q // 128  # 2 question chunks

# float32r matmuls run at full PE rate for N>=256 (fp32 is 1/4 rate) at
# reduced multiply precision (tf32-like). Walrus requires every producer of an
# fp32r-matmul operand to emit fp32r, so the operand tiles are allocated as
# fp32r and exact-fp32 consumers read them through a bitcast view.
USE_F32R = True


def build_nc():
    """Build (trace + schedule + bacc-compile) the single-core Bass program."""
    import concourse.bass as bass
    import concourse.mybir as mybir
    import concourse.tile as tile
    import concourse.bass_isa as bass_isa
    from concourse import bacc
    from concourse.bass import ts
    from concourse.masks import make_identity

    f32 = mybir.dt.float32
    f32r = mybir.dt.float32r
    Alu = mybir.AluOpType
    Act = mybir.ActivationFunctionType
    Axis = mybir.AxisListType

    nc = bacc.Bacc(
        "TRN2",
        target_bir_lowering=False,
        debug=False,
        enable_asserts=False,
        num_devices=N_CORES,
    )

    ep = nc.dram_tensor("encoded_passage", [P, D], f32, kind="ExternalInput").ap()
    eq = nc.dram_tensor("encoded_question", [Q, D], f32, kind="ExternalInput").ap()
    pmsk = nc.dram_tensor("passage_mask", [P], f32, kind="ExternalInput").ap()
    qmsk = nc.dram_tensor("question_mask", [Q], f32, kind="ExternalInput").ap()
    out = nc.dram_tensor("out", [P, 4 * D], f32, kind="ExternalOutput").ap()

    dt_mm = f32r if USE_F32R else f32

    with tile.TileContext(nc) as tc:
        with (
            tc.tile_pool(name="const", bufs=1) as const,
            tc.tile_pool(name="work", bufs=3) as work,
            tc.tile_pool(name="sm", bufs=4) as sm,
            tc.tile_pool(name="small", bufs=4) as small,
            tc.tile_pool(name="store", bufs=4) as store,
            tc.tile_pool(name="psTR", bufs=2, space="PSUM") as psTR,
            tc.tile_pool(name="psSIM", bufs=2, space="PSUM") as psSIM,
            tc.tile_pool(name="psPQ", bufs=1, space="PSUM") as psPQ,
        ):
            # ---- constants / persistent tiles ----
            ident = const.tile([128, 128], f32)
            make_identity(nc, ident)

            pas_all = const.tile([128, NT, D], dt_mm)  # passage, natural layout
            qnat = const.tile([128, QC, D], dt_mm)  # question, natural layout
            qT = const.tile([128, DC, Q], dt_mm)  # question transposed [d, q]
            qmask_b = const.tile([128, Q], f32)  # question mask bcast over rows
            negq_b = const.tile([128, Q], f32)  # NEG_VAL * (1 - qmask)
            qp_all = const.tile([128, NT], f32)  # qp_similarity columns
            r_all = const.tile([128, NT], f32)  # 1/(softmax sum + eps) per tile
            qp_bc = const.tile([128, D], f32)  # qp_vector broadcast

            # question: load natural, broadcast masks
            for qc in range(QC):
                nc.sync.dma_start(
                    out=qnat[:, qc, :], in_=eq[ts(qc, 128), :].bitcast(dt_mm)
                )
            nc.sync.dma_start(out=qmask_b[:, :], in_=qmsk.partition_broadcast(128))
            nc.vector.tensor_scalar(
                out=negq_b[:, :],
                in0=qmask_b[:, :],
                scalar1=-NEG_VAL,
                scalar2=NEG_VAL,
                op0=Alu.mult,
                op1=Alu.add,
            )

            # question transpose: qT[:, dc, qc*128:(qc+1)*128] = qnat[:, qc, dc].T
            qnat_f32 = qnat.bitcast(f32)
            for dc in range(DC):
                ps_q = psTR.tile([128, 8, 128], f32, tag="tr8")
                for qc in range(QC):
                    nc.tensor.transpose(
                        ps_q[:, qc, :],
                        qnat_f32[:, qc, ts(dc, 128)],
                        ident[:, :],
                    )
                nc.vector.tensor_copy(qT[:, dc, :], ps_q[:, 0:2, :])

            # passage mask, transposed to [p_in_tile, tile] layout
            pm_nat = const.tile([16, 128], f32)
            nc.sync.dma_start(
                out=pm_nat[:, :], in_=pmsk.rearrange("(t p) -> t p", p=128)
            )
            ps_pm = psTR.tile([128, 8, 128], f32, tag="tr8")
            nc.tensor.transpose(ps_pm[:, 0, 0:16], pm_nat[:, :], ident[0:16, 0:16])
            pm_t = const.tile([128, NT], f32)
            nc.vector.tensor_copy(pm_t[:, :], ps_pm[:, 0, 0:16])

            # ---- phase 1: per passage-tile attention ----
            for t in range(NT):
                # load passage tile (kept resident for phases 2/3)
                nc.sync.dma_start(
                    out=pas_all[:, t, :], in_=ep[ts(t, 128), :].bitcast(dt_mm)
                )

                # transpose passage tile: pT[:, dc, :] = pas[:, dc-chunk].T
                pas_f32 = pas_all.bitcast(f32)
                pT = work.tile([128, DC, 128], dt_mm, tag="pT")
                ps8 = psTR.tile([128, 8, 128], f32, tag="tr8")
                for dc in range(DC):
                    nc.tensor.transpose(
                        ps8[:, dc, :],
                        pas_f32[:, t, ts(dc, 128)],
                        ident[:, :],
                    )
                nc.vector.tensor_copy(pT[:, :, :], ps8[:, 0:DC, :])

                # sim tile [128, Q] = passage_tile @ question^T
                ps_sim = psSIM.tile([128, Q], f32, tag="sim")
                for dc in range(DC):
                    nc.tensor.matmul(
                        ps_sim[:, :],
                        lhsT=pT[:, dc, :],
                        rhs=qT[:, dc, :],
                        start=(dc == 0),
                        stop=(dc == DC - 1),
                    )

                # im = sim * qmask; negm1 = -max(im)
                im = sm.tile([128, Q], f32, tag="im")
                negm1 = small.tile([128, 1], f32, tag="negm1")
                nc.vector.tensor_mul(im[:, :], ps_sim[:, :], qmask_b[:, :])
                nc.vector.tensor_reduce(
                    out=negm1[:, :], in_=im[:, :], axis=Axis.X, op=Alu.max, negate=True
                )
                # masked_sim = sim*qmask + NEG_VAL*(1-qmask)
                msim = sm.tile([128, Q], f32, tag="msim")
                nc.vector.tensor_add(msim[:, :], im[:, :], negq_b[:, :])
                # qp_similarity column for this tile
                nc.vector.tensor_reduce(
                    out=qp_all[:, t : t + 1],
                    in_=msim[:, :],
                    axis=Axis.X,
                    op=Alu.max,
                )
                # t' = exp(masked_sim - m1)  (masked entries underflow to 0)
                tprime = sm.tile([128, Q], f32, tag="tp")
                ssum = small.tile([128, 1], f32, tag="ssum")
                nc.scalar.activation(
                    out=tprime[:, :],
                    in_=msim[:, :],
                    func=Act.Exp,
                    bias=negm1[:, :],
                    scale=1.0,
                    accum_out=ssum[:, :],
                )
                se = small.tile([128, 1], f32, tag="se")
                nc.vector.tensor_scalar_add(se[:, :], ssum[:, :], EPS)
                nc.vector.reciprocal(r_all[:, t : t + 1], se[:, :])

                # transpose t' -> [q, p] for the pq matmul
                for qc in range(QC):
                    nc.tensor.transpose(
                        ps8[:, DC + qc, :],
                        tprime[:, ts(qc, 128)],
                        ident[:, :],
                    )
                aT = work.tile([128, QC, 128], dt_mm, tag="aT")
                nc.scalar.copy(aT[:, :, :], ps8[:, DC : DC + 2, :])

                # pq_vectors tile [128, D] = t'^T.T @ question  (unnormalized)
                o23 = store.tile([128, 2 * D], f32, tag="o23")
                ps_pqa = psPQ.tile([128, 512], f32, tag="pqa")
                ps_pqb = psPQ.tile([128, 256], f32, tag="pqb")
                for qc in range(QC):
                    st = qc == 0
                    sp = qc == QC - 1
                    nc.tensor.matmul(
                        ps_pqa[:, :],
                        lhsT=aT[:, qc, :],
                        rhs=qnat[:, qc, 0:512],
                        start=st,
                        stop=sp,
                    )
                    nc.tensor.matmul(
                        ps_pqb[:, :],
                        lhsT=aT[:, qc, :],
                        rhs=qnat[:, qc, 512:D],
                        start=st,
                        stop=sp,
                    )

                # evict + normalize pq, form passage*pq, store out cols 768:2304
                nc.scalar.mul(o23[:, 0:512], ps_pqa[:, :], r_all[:, t : t + 1])
                nc.scalar.mul(o23[:, 512:D], ps_pqb[:, :], r_all[:, t : t + 1])
                nc.gpsimd.tensor_mul(o23[:, D : 2 * D], pas_f32[:, t, :], o23[:, 0:D])
                nc.sync.dma_start(out=out[ts(t, 128), D : 3 * D], in_=o23[:, :])

            # ---- phase 2: masked softmax over all P, then qp_vector ----
            neg2 = const.tile([128, NT], f32)
            nc.vector.tensor_scalar(
                out=neg2[:, :],
                in0=pm_t[:, :],
                scalar1=-NEG_VAL,
                scalar2=NEG_VAL,
                op0=Alu.mult,
                op1=Alu.add,
            )
            im2 = const.tile([128, NT], f32)
            nc.vector.tensor_mul(im2[:, :], qp_all[:, :], pm_t[:, :])
            rowmax = small.tile([128, 1], f32, tag="p2")
            nc.vector.tensor_reduce(
                out=rowmax[:, :], in_=im2[:, :], axis=Axis.X, op=Alu.max
            )
            gmax = small.tile([128, 1], f32, tag="p2")
            nc.gpsimd.partition_all_reduce(
                gmax[:, :], rowmax[:, :], channels=128, reduce_op=bass_isa.ReduceOp.max
            )
            neggmax = small.tile([128, 1], f32, tag="p2")
            nc.vector.tensor_scalar_mul(neggmax[:, :], gmax[:, :], -1.0)

            ms2 = const.tile([128, NT], f32)
            nc.vector.tensor_add(ms2[:, :], im2[:, :], neg2[:, :])
            t2 = const.tile([128, NT], f32)
            s2row = small.tile([128, 1], f32, tag="p2")
            nc.scalar.activation(
                out=t2[:, :],
                in_=ms2[:, :],
                func=Act.Exp,
                bias=neggmax[:, :],
                scale=1.0,
                accum_out=s2row[:, :],
            )
            gsum = small.tile([128, 1], f32, tag="p2")
            nc.gpsimd.partition_all_reduce(
                gsum[:, :], s2row[:, :], channels=128, reduce_op=bass_isa.ReduceOp.add
            )
            ser2 = small.tile([128, 1], f32, tag="p2")
            nc.vector.tensor_scalar_add(ser2[:, :], gsum[:, :], EPS)
            r2 = small.tile([128, 1], f32, tag="p2")
            nc.vector.reciprocal(r2[:, :], ser2[:, :])
            attn2 = const.tile([128, NT], dt_mm)
            nc.vector.tensor_scalar_mul(attn2[:, :], t2[:, :], r2[:, :])

            # qp_vector[d] = sum_p attn2[p] * passage[p, d]
            ps_qpa = psPQ.tile([1, 512], f32, tag="pqa")
            ps_qpb = psPQ.tile([1, 256], f32, tag="pqb")
            for t in range(NT):
                st = t == 0
                sp = t == NT - 1
                nc.tensor.matmul(
                    ps_qpa[:, :],
                    lhsT=attn2[:, t : t + 1],
                    rhs=pas_all[:, t, 0:512],
                    start=st,
                    stop=sp,
                )
                nc.tensor.matmul(
                    ps_qpb[:, :],
                    lhsT=attn2[:, t : t + 1],
                    rhs=pas_all[:, t, 512:D],
                    start=st,
                    stop=sp,
                )
            qp_sb = const.tile([1, D], f32)
            nc.scalar.copy(qp_sb[:, 0:512], ps_qpa[:, :])
            nc.scalar.copy(qp_sb[:, 512:D], ps_qpb[:, :])
            nc.gpsimd.partition_broadcast(qp_bc[:, :], qp_sb[:, :], channels=128)

            # ---- phase 3: passage chunk + passage * qp_vector stores ----
            # The passage-chunk stores have no compute dependency; emitting
            # them here fills the DMA gap while the qp chain and the chunk-4
            # muls run.
            pas_f32 = pas_all.bitcast(f32)
            for g in range(NT // 4):
                dst1 = out[g * 512 : (g + 1) * 512, 0:D].rearrange(
                    "(tt p) c -> p tt c", p=128
                )
                nc.sync.dma_start(out=dst1, in_=pas_f32[:, 4 * g : 4 * g + 4, :])
                big = store.tile([128, 4, D], f32, tag="o4")
                for j in range(4):
                    t = 4 * g + j
                    eng = nc.gpsimd if j == 3 else nc.vector
                    eng.tensor_mul(big[:, j, :], pas_f32[:, t, :], qp_bc[:, :])
                dst = out[g * 512 : (g + 1) * 512, 3 * D : 4 * D].rearrange(
                    "(tt p) c -> p tt c", p=128
                )
                nc.sync.dma_start(out=dst, in_=big[:, :, :])

    nc.compile()
    return nc


_NC_CACHE = {}


def _get_nc():
    if "nc" not in _NC_CACHE:
        _NC_CACHE["nc"] = build_nc()
    return _NC_CACHE["nc"]


def kernel(
    encoded_passage: np.ndarray,
    encoded_question: np.ndarray,
    passage_mask: np.ndarray,
    question_mask: np.ndarray,
) -> np.ndarray:
    from concourse.bass_utils import run_bass_kernel_spmd

    nc = _get_nc()
    in_maps = [
        {
            "encoded_passage": np.ascontiguousarray(
                encoded_passage[b], dtype=np.float32
            ),
            "encoded_question": np.ascontiguousarray(
                encoded_question[b], dtype=np.float32
            ),
            "passage_mask": np.ascontiguousarray(passage_mask[b], dtype=np.float32),
            "question_mask": np.ascontiguousarray(question_mask[b], dtype=np.float32),
        }
        for b in range(B)
    ]
    res = run_bass_kernel_spmd(nc, in_maps, core_ids=list(range(N_CORES)))
    return np.stack([res.results[b]["out"] for b in range(B)], axis=0)


# revision 41
# speedup vs baseline: 1.4373x; 1.1770x over previous
"""BiAttention (BiDAF-style) layer for Trainium2, data-parallel over batch.

Shapes (hardcoded, from the problem spec):
  encoded_passage  [B=8, P=2048, D=768] f32
  encoded_question [B=8, Q=256,  D=768] f32
  passage_mask     [B=8, P=2048] f32 (binary)
  question_mask    [B=8, Q=256]  f32 (binary)
  output           [B=8, P=2048, 4*D=3072] f32

Each of the 8 NeuronCores processes one batch element; no communication.
"""

import numpy as np

B, P, Q, D = 8, 2048, 256, 768
N_CORES = 8
EPS = 1e-07
NEG_VAL = -10000000.0

NT = P // 128  # 16 passage tiles
DC = D // 128  # 6 contraction chunks
QC = Q // 128  # 2 question chunks

# float32r matmuls run at full PE rate for N>=256 (fp32 is 1/4 rate) at
# reduced multiply precision (tf32-like). Walrus requires every producer of an
# fp32r-matmul operand to emit fp32r, so the operand tiles are allocated as
# fp32r and exact-fp32 consumers read them through a bitcast view.
USE_F32R = True


def build_nc():
    """Build (trace + schedule + bacc-compile) the single-core Bass program."""
    import concourse.bass as bass
    import concourse.mybir as mybir
    import concourse.tile as tile
    import concourse.bass_isa as bass_isa
    from concourse import bacc
    from concourse.bass import ts
    from concourse.masks import make_identity

    f32 = mybir.dt.float32
    f32r = mybir.dt.float32r
    Alu = mybir.AluOpType
    Act = mybir.ActivationFunctionType
    Axis = mybir.AxisListType

    nc = bacc.Bacc(
        "TRN2",
        target_bir_lowering=False,
        debug=False,
        enable_asserts=False,
        num_devices=N_CORES,
    )

    ep = nc.dram_tensor("encoded_passage", [P, D], f32, kind="ExternalInput").ap()
    eq = nc.dram_tensor("encoded_question", [Q, D], f32, kind="ExternalInput").ap()
    pmsk = nc.dram_tensor("passage_mask", [P], f32, kind="ExternalInput").ap()
    qmsk = nc.dram_tensor("question_mask", [Q], f32, kind="ExternalInput").ap()
    out = nc.dram_tensor("out", [P, 4 * D], f32, kind="ExternalOutput").ap()

    dt_mm = f32r if USE_F32R else f32

    with tile.TileContext(nc) as tc:
        with (
            tc.tile_pool(name="const", bufs=1) as const,
            tc.tile_pool(name="work", bufs=5) as work,
            tc.tile_pool(name="sm", bufs=6) as sm,
            tc.tile_pool(name="small", bufs=4) as small,
            tc.tile_pool(name="store", bufs=6) as store,
            tc.tile_pool(name="psTR", bufs=2, space="PSUM") as psTR,
            tc.tile_pool(name="psSIM", bufs=2, space="PSUM") as psSIM,
            tc.tile_pool(name="psPQ", bufs=1, space="PSUM") as psPQ,
        ):
            # ---- constants / persistent tiles ----
            ident = const.tile([128, 128], f32)
            make_identity(nc, ident)

            pas_all = const.tile([128, NT, D], dt_mm)  # passage, natural layout
            qnat = const.tile([128, QC, D], dt_mm)  # question, natural layout
            qT = const.tile([128, DC, Q], dt_mm)  # qmask * question^T  [d, q]
            qmask_b = const.tile([128, Q], f32)  # question mask bcast over rows
            qp_all = const.tile([128, NT], f32)  # qp_similarity columns
            r_all = const.tile([128, NT], f32)  # 1/(softmax sum + eps) per tile
            qp_bc = const.tile([128, D], f32)  # qp_vector broadcast
            ones_l = const.tile([1, 128], dt_mm)  # K=1 lhsT for the mask row
            negq_row = const.tile([1, Q], dt_mm)  # NEG_VAL * (1 - qmask)

            # question: load natural, broadcast masks
            for qc in range(QC):
                nc.sync.dma_start(
                    out=qnat[:, qc, :], in_=eq[ts(qc, 128), :].bitcast(dt_mm)
                )
            nc.sync.dma_start(out=qmask_b[:, :], in_=qmsk.partition_broadcast(128))
            ones_f = const.tile([1, 128], f32)
            nc.vector.memset(ones_f[:, :], 1.0)
            nc.vector.tensor_copy(ones_l[:, :], ones_f[:, :])
            nc.vector.tensor_scalar(
                out=negq_row[:, :],
                in0=qmask_b[0:1, :],
                scalar1=-NEG_VAL,
                scalar2=NEG_VAL,
                op0=Alu.mult,
                op1=Alu.add,
            )

            # masked question transpose: qT[:, dc, :] = qmask * qnat[:, :, dc].T
            # Folding the mask here makes the sim matmul emit qmask*sim, and
            # the extra ones_l x negq_row rank-1 matmul adds NEG_VAL*(1-qmask)
            # so PSUM holds masked_sim directly.
            qnat_f32 = qnat.bitcast(f32)
            for dc in range(DC):
                ps_q = psTR.tile([128, 8, 128], f32, tag="tr8")
                for qc in range(QC):
                    nc.tensor.transpose(
                        ps_q[:, qc, :],
                        qnat_f32[:, qc, ts(dc, 128)],
                        ident[:, :],
                    )
                nc.vector.tensor_mul(
                    qT[:, dc, :],
                    ps_q.rearrange("p a b -> p (a b)")[:, 0:Q],
                    qmask_b[:, :],
                )

            # passage mask, transposed to [p_in_tile, tile] layout
            pm_nat = const.tile([16, 128], f32)
            nc.sync.dma_start(
                out=pm_nat[:, :], in_=pmsk.rearrange("(t p) -> t p", p=128)
            )
            ps_pm = psTR.tile([128, 8, 128], f32, tag="tr8")
            nc.tensor.transpose(ps_pm[:, 0, 0:16], pm_nat[:, :], ident[0:16, 0:16])
            pm_t = const.tile([128, NT], f32)
            nc.vector.tensor_copy(pm_t[:, :], ps_pm[:, 0, 0:16])

            # ---- phase 1: per passage-tile attention ----
            for t in range(NT):
                # load passage tile (kept resident for phases 2/3)
                nc.sync.dma_start(
                    out=pas_all[:, t, :], in_=ep[ts(t, 128), :].bitcast(dt_mm)
                )

                # transpose passage tile: pT[:, dc, :] = pas[:, dc-chunk].T
                pas_f32 = pas_all.bitcast(f32)
                pT = work.tile([128, DC, 128], dt_mm, tag="pT")
                ps8 = psTR.tile([128, 8, 128], f32, tag="tr8")
                for dc in range(DC):
                    nc.tensor.transpose(
                        ps8[:, dc, :],
                        pas_f32[:, t, ts(dc, 128)],
                        ident[:, :],
                    )
                nc.vector.tensor_copy(pT[:, :, :], ps8[:, 0:DC, :])

                # masked_sim tile [128, Q] in PSUM:
                #   qmask*(passage @ question^T) + NEG_VAL*(1-qmask)
                ps_sim = psSIM.tile([128, Q], f32, tag="sim")
                for dc in range(DC):
                    nc.tensor.matmul(
                        ps_sim[:, :],
                        lhsT=pT[:, dc, :],
                        rhs=qT[:, dc, :],
                        start=(dc == 0),
                        stop=False,
                    )
                nc.tensor.matmul(
                    ps_sim[:, :],
                    lhsT=ones_l[:, :],
                    rhs=negq_row[:, :],
                    start=False,
                    stop=True,
                )

                # max(masked_sim) is both the softmax shift and qp_similarity
                negm1 = small.tile([128, 1], f32, tag="negm1")
                nc.vector.tensor_reduce(
                    out=negm1[:, :],
                    in_=ps_sim[:, :],
                    axis=Axis.X,
                    op=Alu.max,
                    negate=True,
                )
                nc.vector.tensor_scalar_mul(qp_all[:, t : t + 1], negm1[:, :], -1.0)
                # t' = exp(masked_sim - m1)  (masked entries underflow to 0)
                tprime = sm.tile([128, Q], f32, tag="tp")
                ssum = small.tile([128, 1], f32, tag="ssum")
                nc.scalar.activation(
                    out=tprime[:, :],
                    in_=ps_sim[:, :],
                    func=Act.Exp,
                    bias=negm1[:, :],
                    scale=1.0,
                    accum_out=ssum[:, :],
                )
                se = small.tile([128, 1], f32, tag="se")
                nc.vector.tensor_scalar_add(se[:, :], ssum[:, :], EPS)
                nc.vector.reciprocal(r_all[:, t : t + 1], se[:, :])

                # transpose t' -> [q, p] for the pq matmul
                for qc in range(QC):
                    nc.tensor.transpose(
                        ps8[:, DC + qc, :],
                        tprime[:, ts(qc, 128)],
                        ident[:, :],
                    )
                aT = work.tile([128, QC, 128], dt_mm, tag="aT")
                nc.scalar.copy(aT[:, :, :], ps8[:, DC : DC + 2, :])

                # pq_vectors tile [128, D] = t'^T.T @ question  (unnormalized)
                o23 = store.tile([128, 2 * D], f32, tag="o23")
                ps_pqa = psPQ.tile([128, 512], f32, tag="pqa")
                ps_pqb = psPQ.tile([128, 256], f32, tag="pqb")
                for qc in range(QC):
                    st = qc == 0
                    sp = qc == QC - 1
                    nc.tensor.matmul(
                        ps_pqa[:, :],
                        lhsT=aT[:, qc, :],
                        rhs=qnat[:, qc, 0:512],
                        start=st,
                        stop=sp,
                    )
                    nc.tensor.matmul(
                        ps_pqb[:, :],
                        lhsT=aT[:, qc, :],
                        rhs=qnat[:, qc, 512:D],
                        start=st,
                        stop=sp,
                    )

                # evict + normalize pq, form passage*pq, store out cols 768:2304
                nc.scalar.mul(o23[:, 0:512], ps_pqa[:, :], r_all[:, t : t + 1])
                nc.scalar.mul(o23[:, 512:D], ps_pqb[:, :], r_all[:, t : t + 1])
                nc.gpsimd.tensor_mul(o23[:, D : 2 * D], pas_f32[:, t, :], o23[:, 0:D])
                nc.sync.dma_start(out=out[ts(t, 128), D : 3 * D], in_=o23[:, :])

            # ---- phase 2: masked softmax over all P, then qp_vector ----
            neg2 = const.tile([128, NT], f32)
            nc.vector.tensor_scalar(
                out=neg2[:, :],
                in0=pm_t[:, :],
                scalar1=-NEG_VAL,
                scalar2=NEG_VAL,
                op0=Alu.mult,
                op1=Alu.add,
            )
            im2 = const.tile([128, NT], f32)
            nc.vector.tensor_mul(im2[:, :], qp_all[:, :], pm_t[:, :])
            rowmax = small.tile([128, 1], f32, tag="p2")
            nc.vector.tensor_reduce(
                out=rowmax[:, :], in_=im2[:, :], axis=Axis.X, op=Alu.max
            )
            gmax = small.tile([128, 1], f32, tag="p2")
            nc.gpsimd.partition_all_reduce(
                gmax[:, :], rowmax[:, :], channels=128, reduce_op=bass_isa.ReduceOp.max
            )
            neggmax = small.tile([128, 1], f32, tag="p2")
            nc.vector.tensor_scalar_mul(neggmax[:, :], gmax[:, :], -1.0)

            ms2 = const.tile([128, NT], f32)
            nc.vector.tensor_add(ms2[:, :], im2[:, :], neg2[:, :])
            t2 = const.tile([128, NT], f32)
            s2row = small.tile([128, 1], f32, tag="p2")
            nc.scalar.activation(
                out=t2[:, :],
                in_=ms2[:, :],
                func=Act.Exp,
                bias=neggmax[:, :],
                scale=1.0,
                accum_out=s2row[:, :],
            )
            gsum = small.tile([128, 1], f32, tag="p2")
            nc.gpsimd.partition_all_reduce(
                gsum[:, :], s2row[:, :], channels=128, reduce_op=bass_isa.ReduceOp.add
            )
            ser2 = small.tile([128, 1], f32, tag="p2")
            nc.vector.tensor_scalar_add(ser2[:, :], gsum[:, :], EPS)
            r2 = small.tile([128, 1], f32, tag="p2")
            nc.vector.reciprocal(r2[:, :], ser2[:, :])
            attn2 = const.tile([128, NT], dt_mm)
            nc.vector.tensor_scalar_mul(attn2[:, :], t2[:, :], r2[:, :])

            # qp_vector[d] = sum_p attn2[p] * passage[p, d]
            ps_qpa = psPQ.tile([1, 512], f32, tag="pqa")
            ps_qpb = psPQ.tile([1, 256], f32, tag="pqb")
            for t in range(NT):
                st = t == 0
                sp = t == NT - 1
                nc.tensor.matmul(
                    ps_qpa[:, :],
                    lhsT=attn2[:, t : t + 1],
                    rhs=pas_all[:, t, 0:512],
                    start=st,
                    stop=sp,
                )
                nc.tensor.matmul(
                    ps_qpb[:, :],
                    lhsT=attn2[:, t : t + 1],
                    rhs=pas_all[:, t, 512:D],
                    start=st,
                    stop=sp,
                )
            qp_sb = const.tile([1, D], f32)
            nc.scalar.copy(qp_sb[:, 0:512], ps_qpa[:, :])
            nc.scalar.copy(qp_sb[:, 512:D], ps_qpb[:, :])
            nc.gpsimd.partition_broadcast(qp_bc[:, :], qp_sb[:, :], channels=128)

            # ---- phase 3: passage chunk + passage * qp_vector stores ----
            # The passage-chunk stores have no compute dependency; emitting
            # them here fills the DMA gap while the qp chain and the chunk-4
            # muls run.
            pas_f32 = pas_all.bitcast(f32)
            for g in range(NT // 4):
                dst1 = out[g * 512 : (g + 1) * 512, 0:D].rearrange(
                    "(tt p) c -> p tt c", p=128
                )
                nc.sync.dma_start(out=dst1, in_=pas_f32[:, 4 * g : 4 * g + 4, :])
                big = store.tile([128, 4, D], f32, tag="o4")
                for j in range(4):
                    t = 4 * g + j
                    eng = nc.gpsimd if j == 3 else nc.vector
                    eng.tensor_mul(big[:, j, :], pas_f32[:, t, :], qp_bc[:, :])
                dst = out[g * 512 : (g + 1) * 512, 3 * D : 4 * D].rearrange(
                    "(tt p) c -> p tt c", p=128
                )
                nc.sync.dma_start(out=dst, in_=big[:, :, :])

    nc.compile()
    return nc


_NC_CACHE = {}


def _get_nc():
    if "nc" not in _NC_CACHE:
        _NC_CACHE["nc"] = build_nc()
    return _NC_CACHE["nc"]


def kernel(
    encoded_passage: np.ndarray,
    encoded_question: np.ndarray,
    passage_mask: np.ndarray,
    question_mask: np.ndarray,
) -> np.ndarray:
    from concourse.bass_utils import run_bass_kernel_spmd

    nc = _get_nc()
    in_maps = [
        {
            "encoded_passage": np.ascontiguousarray(
                encoded_passage[b], dtype=np.float32
            ),
            "encoded_question": np.ascontiguousarray(
                encoded_question[b], dtype=np.float32
            ),
            "passage_mask": np.ascontiguousarray(passage_mask[b], dtype=np.float32),
            "question_mask": np.ascontiguousarray(question_mask[b], dtype=np.float32),
        }
        for b in range(B)
    ]
    res = run_bass_kernel_spmd(nc, in_maps, core_ids=list(range(N_CORES)))
    return np.stack([res.results[b]["out"] for b in range(B)], axis=0)
